# revision 1
# baseline (speedup 1.0000x reference)
"""PointPillarScatter (intersweep, 3 bins) Trainium2 Bass kernel.

Problem: for each of 3 bins, scatter 64000 pillar rows [64 feats] into a
[B=4, C=64, NY=496, NX=432] canvas at (b, :, y, x); empty cells are zero.

Strategy (8 NeuronCores, SPMD):
  - The output is 658 MB, ~92.5% zeros -> the kernel is write-bandwidth
    bound. We generate every output tile densely on-chip and store it with
    large contiguous DMAs.
  - Host-side (cheap numpy): shard the 12 (bin, b) canvases into 48
    quarter-canvases of 124 y-rows; 6 per core, processed as 3 pairs.
    For each y-row ("window", 432 cells) collect its pillars (max 36),
    padded to RPAD slots.
  - On-device, per window, a single fp32 matmul places the pillar features:
      out[128, 432] = lhsT[K=2*RPAD, 128].T @ onehot[K, 432]
    where lhsT is block-diagonal (quarter A's padded pillar features in
    rows 0:RPAD x cols 0:64, quarter B's in rows RPAD:2*RPAD x cols 64:128)
    and onehot[k, c] = (x_rel[k] == c), built by one DVE tensor_scalar
    is_equal against an iota row. The matmul writes zeros everywhere else,
    so dense tiles (zeros included) are produced at PE speed. fp32 one-hot
    matmul is bit-exact on TRN2 (verified on HW).
  - PSUM tiles are copied to an SBUF staging buffer (31 windows deep) and
    written out as [64 part x 53568 B] contiguous DMAs.
"""

import numpy as np

import concourse.bass as bass
import concourse.tile as tile
from concourse import bacc, mybir
from concourse.bass_utils import run_bass_kernel_spmd

# Problem geometry (hardcoded; kernel.py must be self-contained).
B = 4
C = 64
NX = 432
NY = 496
NBINS = 3
P_PER = 16000
NCORES = 8

NQ = NBINS * B * 4  # 48 quarter-canvases
YQ = NY // 4  # 124 y-rows per quarter
QPC = NQ // NCORES  # 6 quarters per core
PAIRS = QPC // 2  # 3 pairs per core
WINDOWS = PAIRS * YQ  # 372 windows per core
CHUNK = 31  # windows per staging chunk
NCHUNKS = YQ // CHUNK  # 4 chunks per pair

_cache = {}


def _build(rpad):
    """Build the SPMD Bass program. Inputs (per core):
    lhst [2*rpad, WINDOWS, 128] f32  (block-diagonal stationary operands)
    meta [2*rpad, NX + WINDOWS] f32  (iota row | per-window rel cells)
    Output: out [QPC, C, YQ, NX] f32.
    """
    K = 2 * rpad
    nc = bacc.Bacc(trn_type="TRN2")
    lhst_d = nc.dram_tensor("lhst", [K, WINDOWS, 128], mybir.dt.float32,
                            kind="ExternalInput")
    meta_d = nc.dram_tensor("meta", [K, NX + WINDOWS], mybir.dt.float32,
                            kind="ExternalInput")
    out_d = nc.dram_tensor("out", [QPC, C, YQ, NX], mybir.dt.float32,
                           kind="ExternalOutput")

    with tile.TileContext(nc) as tc:
        with (
            tc.tile_pool(name="const", bufs=1) as constp,
            tc.tile_pool(name="lhstp", bufs=2) as lhstp,
            tc.tile_pool(name="maskp", bufs=3) as maskp,
            tc.tile_pool(name="stage", bufs=2) as stagep,
            tc.tile_pool(name="psum", bufs=4, space=bass.MemorySpace.PSUM) as psump,
        ):
            meta = constp.tile([K, NX + WINDOWS], mybir.dt.float32)
            nc.sync.dma_start(out=meta[:], in_=meta_d[:])
            for pair in range(PAIRS):
                for ch in range(NCHUNKS):
                    g0 = pair * YQ + ch * CHUNK
                    lt = lhstp.tile([K, CHUNK, 128], mybir.dt.float32)
                    nc.sync.dma_start(out=lt[:], in_=lhst_d[:, g0:g0 + CHUNK, :])
                    st = stagep.tile([128, CHUNK, NX], mybir.dt.float32)
                    for w in range(CHUNK):
                        mask = maskp.tile([K, NX], mybir.dt.float32)
                        nc.vector.tensor_scalar(
                            out=mask[:],
                            in0=meta[:, 0:NX],
                            scalar1=meta[:, NX + g0 + w:NX + g0 + w + 1],
                            scalar2=None,
                            op0=mybir.AluOpType.is_equal,
                        )
                        acc = psump.tile([128, NX], mybir.dt.float32)
                        nc.tensor.matmul(acc[:], lt[:, w, :], mask[:],
                                         start=True, stop=True)
                        # alternate PSUM->SBUF copies between DVE and ACT
                        if (ch * CHUNK + w) % 2 == 0:
                            nc.vector.tensor_copy(out=st[:, w, :], in_=acc[:])
                        else:
                            nc.scalar.copy(st[:, w, :], acc[:])
                    c0 = ch * CHUNK
                    nc.sync.dma_start(
                        out=out_d[2 * pair, :, c0:c0 + CHUNK, :],
                        in_=st[0:C],
                    )
                    nc.sync.dma_start(
                        out=out_d[2 * pair + 1, :, c0:c0 + CHUNK, :],
                        in_=st[C:128],
                    )
    nc.compile()
    return nc


def _pack(inputs, rpad):
    """Host-side packing: returns in_maps (list per core)."""
    K = 2 * rpad
    lhst = np.zeros((NCORES, K, WINDOWS, 128), np.float32)
    meta = np.zeros((NCORES, K, NX + WINDOWS), np.float32)
    meta[:, :, 0:NX] = np.arange(NX, dtype=np.float32)[None, None, :]
    meta[:, :, NX:] = -1.0

    for bin_i in range(NBINS):
        feats = np.asarray(inputs[f"pillar_features_bin_{bin_i}"], np.float32)
        coords = np.asarray(inputs[f"voxel_coords_bin_{bin_i}"])
        cb = np.asarray(coords[:, 0], np.int64)
        cy = np.asarray(coords[:, 2], np.int64)
        cx = np.asarray(coords[:, 3], np.int64)
        for b in range(B):
            rows_b = np.nonzero(cb == b)[0]
            y_b, x_b = cy[rows_b], cx[rows_b]
            for yq in range(4):
                q = bin_i * 16 + b * 4 + yq
                core, j = divmod(q, QPC)
                pair, half = divmod(j, 2)
                sel = (y_b >= YQ * yq) & (y_b < YQ * (yq + 1))
                rows = rows_b[sel]
                yl = y_b[sel] - YQ * yq
                xs = x_b[sel]
                order = np.argsort(yl, kind="stable")
                rows, yl, xs = rows[order], yl[order], xs[order]
                cnt = np.bincount(yl, minlength=YQ)
                if cnt.max() > rpad:
                    raise OverflowError(int(cnt.max()))
                off = np.concatenate([[0], np.cumsum(cnt)[:-1]])
                slot = np.arange(len(rows)) - off[yl]
                wins = pair * YQ + yl
                srow = slot + half * rpad
                mcol0 = half * C
                lhst[core, srow, wins, mcol0:mcol0 + C] = feats[rows]
                meta[core, srow, NX + wins] = xs.astype(np.float32)
    return [{"lhst": lhst[c], "meta": meta[c]} for c in range(NCORES)]


def _run(inputs, rpad, trace=False):
    key = rpad
    if key not in _cache:
        _cache[key] = _build(rpad)
    nc = _cache[key]
    in_maps = _pack(inputs, rpad)
    res = run_bass_kernel_spmd(nc, in_maps, core_ids=list(range(NCORES)),
                               trace=trace)
    outs = [np.zeros((B, C, NY, NX), np.float32) for _ in range(NBINS)]
    for q in range(NQ):
        bin_i, rem = divmod(q, 16)
        b, yq = divmod(rem, 4)
        core, j = divmod(q, QPC)
        outs[bin_i][b, :, YQ * yq:YQ * (yq + 1), :] = res.results[core]["out"][j]
    return tuple(outs), res


def kernel(**inputs):
    rpad = 40
    while True:
        try:
            out, _ = _run(inputs, rpad)
            return out
        except OverflowError as e:
            rpad = (int(e.args[0]) + 7) // 8 * 8


def kernel_traced(**inputs):
    """Like kernel() but also returns BassKernelResults (for test.py)."""
    return _run(inputs, 40, trace=True)


# revision 3
# speedup vs baseline: 1.2200x; 1.2200x over previous
"""PointPillarScatter (intersweep, 3 bins) Trainium2 Bass kernel.

Problem: for each of 3 bins, scatter 64000 pillar rows [64 feats] into a
[B=4, C=64, NY=496, NX=432] canvas at (b, :, y, x); empty cells are zero.

Strategy (8 NeuronCores, SPMD):
  - The output is 658 MB, ~92.5% zeros -> write-bandwidth bound. Dense
    output tiles (zeros included) are generated on-chip and stored with
    large contiguous DMAs.
  - Host-side (cheap numpy): shard the 12 (bin, b) canvases into 48
    quarter-canvases of 124 y-rows; 6 per core, processed as 3 pairs.
    For each y-row ("window", 432 cells) collect its pillars (max 36),
    padded to RPAD slots.
  - On-device, per window, matmuls place the pillar features:
      out[128, 432] = lhsT[K=2*RPAD, 128].T @ onehot[K, 432]
    lhsT is block-diagonal (quarter A's padded pillar features in rows
    0:RPAD x cols 0:64, quarter B's in rows RPAD:2*RPAD x cols 64:128);
    onehot[k, c] = (x_rel[k] == c) built by one DVE tensor_scalar
    is_equal against an iota row.
  - fp32 matmul runs at ~4 cyc/row on PE, so features are split into
    three bf16 terms (h1+h2+h3 == fp32 value exactly for normal-range
    floats; verified) and placed by 3 accumulating bf16 matmuls at
    1 cyc/row. Result is bit-exact.
  - Only the diagonal blocks are DMA'd (compact [40, w, 64] loads into
    persistent pre-zeroed SBUF tiles); off-diagonal zeros are never
    rewritten.
  - PSUM tiles are copied to SBUF staging (DVE/ACT split) and written out
    as one [128 part x 53568 B] contiguous DMA per 31-window chunk; the
    host de-interleaves the two quarters from the partition halves.
"""

import numpy as np
import ml_dtypes

import concourse.bass as bass
import concourse.tile as tile
from concourse import bacc, mybir
from concourse.bass_utils import run_bass_kernel_spmd

# Problem geometry (hardcoded; kernel.py must be self-contained).
B = 4
C = 64
NX = 432
NY = 496
NBINS = 3
NCORES = 8

NQ = NBINS * B * 4  # 48 quarter-canvases
YQ = NY // 4  # 124 y-rows per quarter
QPC = NQ // NCORES  # 6 quarters per core
PAIRS = QPC // 2  # 3 pairs per core
WINDOWS = PAIRS * YQ  # 372 windows per core
CHUNK = 31  # windows per staging chunk
NCHUNKS = YQ // CHUNK  # 4 chunks per pair
NTERMS = 3  # bf16 split terms (exact fp32 reconstruction)
RPAD = 40  # padded pillar slots per window per quarter (max count is 36)

# fraction of PSUM->SBUF copies on DVE (rest on ACT); masks are on DVE
DVE_COPY_EVERY = 8

_cache = {}


def _build(rpad):
    K = 2 * rpad
    nc = bacc.Bacc(trn_type="TRN2")
    bf16 = mybir.dt.bfloat16
    f32 = mybir.dt.float32
    lhst_d = nc.dram_tensor("lhst", [NTERMS, 2, rpad, WINDOWS, C], bf16,
                            kind="ExternalInput")
    meta_d = nc.dram_tensor("meta", [K, NX + WINDOWS], f32,
                            kind="ExternalInput")
    out_d = nc.dram_tensor("out", [PAIRS, NCHUNKS, 128, CHUNK, NX], f32,
                           kind="ExternalOutput")

    with tile.TileContext(nc) as tc:
        with (
            tc.tile_pool(name="const", bufs=1) as constp,
            tc.tile_pool(name="lhstp", bufs=1) as lhstp,
            tc.tile_pool(name="maskp", bufs=3) as maskp,
            tc.tile_pool(name="stage", bufs=2) as stagep,
            tc.tile_pool(name="psum", bufs=4, space=bass.MemorySpace.PSUM) as psump,
        ):
            meta = constp.tile([K, NX + WINDOWS], f32)
            nc.sync.dma_start(out=meta[:], in_=meta_d[:])
            # persistent ping-pong stationary tiles; off-diagonal blocks are
            # memset to zero once and never rewritten
            lts = [[lhstp.tile([K, CHUNK, 128], bf16, name=f"lt{t}{pp}",
                               tag=f"lt{t}{pp}")
                    for pp in range(2)] for t in range(NTERMS)]
            for t in range(NTERMS):
                for pp in range(2):
                    nc.gpsimd.memset(lts[t][pp][:], 0.0)
            gw = 0
            for pair in range(PAIRS):
                for ch in range(NCHUNKS):
                    g0 = pair * YQ + ch * CHUNK
                    pp = (pair * NCHUNKS + ch) % 2
                    for t in range(NTERMS):
                        lt = lts[t][pp]
                        nc.sync.dma_start(
                            out=lt[0:rpad, :, 0:C],
                            in_=lhst_d[t, 0, :, g0:g0 + CHUNK, :])
                        nc.sync.dma_start(
                            out=lt[rpad:K, :, C:128],
                            in_=lhst_d[t, 1, :, g0:g0 + CHUNK, :])
                    st = stagep.tile([128, CHUNK, NX], f32)
                    for w in range(CHUNK):
                        mask = maskp.tile([K, NX], bf16)
                        nc.vector.tensor_scalar(
                            out=mask[:],
                            in0=meta[:, 0:NX],
                            scalar1=meta[:, NX + g0 + w:NX + g0 + w + 1],
                            scalar2=None,
                            op0=mybir.AluOpType.is_equal,
                        )
                        acc = psump.tile([128, NX], f32)
                        for t in range(NTERMS):
                            nc.tensor.matmul(
                                acc[:], lts[t][pp][:, w, :], mask[:],
                                start=(t == 0), stop=(t == NTERMS - 1))
                        if gw % DVE_COPY_EVERY == 0:
                            nc.vector.tensor_copy(out=st[:, w, :], in_=acc[:])
                        else:
                            nc.scalar.copy(st[:, w, :], acc[:])
                        gw += 1
                    nc.sync.dma_start(out=out_d[pair, ch], in_=st[:])
    nc.compile()
    return nc


def _split3(feats):
    """Split fp32 features into 3 bf16 terms summing exactly to the input."""
    h1 = feats.astype(ml_dtypes.bfloat16)
    r1 = feats - h1.astype(np.float32)
    h2 = r1.astype(ml_dtypes.bfloat16)
    h3 = (r1 - h2.astype(np.float32)).astype(ml_dtypes.bfloat16)
    rec = (h1.astype(np.float32) + h2.astype(np.float32)) + h3.astype(np.float32)
    if not (rec == feats).all():
        raise FloatingPointError("bf16 3-term split not exact")
    return h1, h2, h3


def _pack(inputs, rpad):
    lhst = np.zeros((NCORES, NTERMS, 2, rpad, WINDOWS, C), ml_dtypes.bfloat16)
    meta = np.zeros((NCORES, 2 * rpad, NX + WINDOWS), np.float32)
    meta[:, :, 0:NX] = np.arange(NX, dtype=np.float32)[None, None, :]
    meta[:, :, NX:] = -1.0

    for bin_i in range(NBINS):
        feats = np.asarray(inputs[f"pillar_features_bin_{bin_i}"], np.float32)
        terms = _split3(feats)
        coords = np.asarray(inputs[f"voxel_coords_bin_{bin_i}"])
        cb = np.asarray(coords[:, 0], np.int64)
        cy = np.asarray(coords[:, 2], np.int64)
        cx = np.asarray(coords[:, 3], np.int64)
        for b in range(B):
            rows_b = np.nonzero(cb == b)[0]
            y_b, x_b = cy[rows_b], cx[rows_b]
            for yq in range(4):
                q = bin_i * 16 + b * 4 + yq
                core, j = divmod(q, QPC)
                pair, half = divmod(j, 2)
                sel = (y_b >= YQ * yq) & (y_b < YQ * (yq + 1))
                rows = rows_b[sel]
                yl = y_b[sel] - YQ * yq
                xs = x_b[sel]
                order = np.argsort(yl, kind="stable")
                rows, yl, xs = rows[order], yl[order], xs[order]
                cnt = np.bincount(yl, minlength=YQ)
                if cnt.max() > rpad:
                    raise OverflowError(int(cnt.max()))
                off = np.concatenate([[0], np.cumsum(cnt)[:-1]])
                slot = np.arange(len(rows)) - off[yl]
                wins = pair * YQ + yl
                for t in range(NTERMS):
                    lhst[core, t, half, slot, wins, :] = terms[t][rows]
                meta[core, slot + half * rpad, NX + wins] = xs.astype(np.float32)
    return [{"lhst": lhst[c], "meta": meta[c]} for c in range(NCORES)]


def _run(inputs, rpad, trace=False):
    if rpad not in _cache:
        _cache[rpad] = _build(rpad)
    nc = _cache[rpad]
    in_maps = _pack(inputs, rpad)
    res = run_bass_kernel_spmd(nc, in_maps, core_ids=list(range(NCORES)),
                               trace=trace)
    outs = [np.zeros((B, C, NY, NX), np.float32) for _ in range(NBINS)]
    for q in range(NQ):
        bin_i, rem = divmod(q, 16)
        b, yq = divmod(rem, 4)
        core, j = divmod(q, QPC)
        pair, half = divmod(j, 2)
        # [NCHUNKS, 64, CHUNK, NX] -> [64, NCHUNKS*CHUNK, NX]
        blk = res.results[core]["out"][pair, :, half * C:(half + 1) * C]
        outs[bin_i][b, :, YQ * yq:YQ * (yq + 1), :] = (
            blk.transpose(1, 0, 2, 3).reshape(C, YQ, NX))
    return tuple(outs), res


def kernel(**inputs):
    rpad = RPAD
    while True:
        try:
            out, _ = _run(inputs, rpad)
            return out
        except OverflowError as e:
            rpad = (int(e.args[0]) + 7) // 8 * 8


def kernel_traced(**inputs):
    """Like kernel() but also returns BassKernelResults (for test.py)."""
    return _run(inputs, RPAD, trace=True)


# revision 4
# speedup vs baseline: 1.6176x; 1.3260x over previous
"""PointPillarScatter (intersweep, 3 bins) Trainium2 Bass kernel.

Problem: for each of 3 bins, scatter 64000 pillar rows [64 feats] into a
[B=4, C=64, NY=496, NX=432] canvas at (b, :, y, x); empty cells are zero.

Strategy (8 NeuronCores, SPMD):
  - The output is 658 MB, ~92.5% zeros -> write-bandwidth bound. Dense
    output tiles (zeros included) are generated on-chip and stored with
    large contiguous DMAs.
  - Host-side (cheap numpy): shard the 12 (bin, b) canvases into 48
    quarter-canvases of 124 y-rows; 6 per core, processed as 3 pairs.
    For each y-row ("window", 432 cells) collect its pillars (max 36),
    padded to RPAD slots.
  - On-device, per window, matmuls place the pillar features:
      out[128, 432] = lhsT[K=2*RPAD, 128].T @ onehot[K, 432]
    lhsT is block-diagonal (quarter A's padded pillar features in rows
    0:RPAD x cols 0:64, quarter B's in rows RPAD:2*RPAD x cols 64:128);
    onehot[k, c] = (x_rel[k] == c) built by one DVE tensor_scalar
    is_equal against an iota row.
  - fp32 matmul runs at ~4 cyc/row on PE, so features are split into
    three bf16 terms (h1+h2+h3 == fp32 value exactly for normal-range
    floats; verified) and placed by 3 accumulating bf16 matmuls at
    1 cyc/row. Result is bit-exact.
  - Only the diagonal blocks are DMA'd (compact [40, w, 64] loads into
    persistent pre-zeroed SBUF tiles); off-diagonal zeros are never
    rewritten.
  - PSUM tiles are copied to SBUF staging (DVE/ACT split) and written out
    as one [128 part x 53568 B] contiguous DMA per 31-window chunk; the
    host de-interleaves the two quarters from the partition halves.
"""

import numpy as np
import ml_dtypes

import concourse.bass as bass
import concourse.tile as tile
from concourse import bacc, mybir
from concourse.bass_utils import run_bass_kernel_spmd

# Problem geometry (hardcoded; kernel.py must be self-contained).
B = 4
C = 64
NX = 432
NY = 496
NBINS = 3
NCORES = 8

NQ = NBINS * B * 4  # 48 quarter-canvases
YQ = NY // 4  # 124 y-rows per quarter
QPC = NQ // NCORES  # 6 quarters per core
PAIRS = QPC // 2  # 3 pairs per core
WINDOWS = PAIRS * YQ  # 372 windows per core
CHUNK = 31  # windows per staging chunk
NCHUNKS = YQ // CHUNK  # 4 chunks per pair
NTERMS = 3  # bf16 split terms (exact fp32 reconstruction)
RPAD = 40  # padded pillar slots per window per quarter (max count is 36)

# fraction of PSUM->SBUF copies on DVE (rest on ACT); masks are on DVE
DVE_COPY_EVERY = 8

_cache = {}


def _build(rpad):
    K = 128  # full partition dim: K<128 matmuls run at half rate on TRN2
    nc = bacc.Bacc(trn_type="TRN2")
    bf16 = mybir.dt.bfloat16
    f32 = mybir.dt.float32
    lhst_d = nc.dram_tensor("lhst", [NTERMS, 2, rpad, WINDOWS, C], bf16,
                            kind="ExternalInput")
    meta_d = nc.dram_tensor("meta", [K, NX + WINDOWS], f32,
                            kind="ExternalInput")
    out_d = nc.dram_tensor("out", [PAIRS, NCHUNKS, 128, CHUNK, NX], f32,
                           kind="ExternalOutput")

    with tile.TileContext(nc) as tc:
        with (
            tc.tile_pool(name="const", bufs=1) as constp,
            tc.tile_pool(name="lhstp", bufs=1) as lhstp,
            tc.tile_pool(name="maskp", bufs=3) as maskp,
            tc.tile_pool(name="stage", bufs=2) as stagep,
            tc.tile_pool(name="psum", bufs=4, space=bass.MemorySpace.PSUM) as psump,
        ):
            meta = constp.tile([K, NX + WINDOWS], f32)
            nc.gpsimd.dma_start(out=meta[:], in_=meta_d[:])
            # persistent ping-pong stationary tiles; off-diagonal blocks are
            # memset to zero once and never rewritten
            lts = [[lhstp.tile([K, CHUNK, 128], bf16, name=f"lt{t}{pp}",
                               tag=f"lt{t}{pp}")
                    for pp in range(2)] for t in range(NTERMS)]
            for t in range(NTERMS):
                for pp in range(2):
                    nc.gpsimd.memset(lts[t][pp][:], 0.0)
            gw = 0
            for pair in range(PAIRS):
                for ch in range(NCHUNKS):
                    g0 = pair * YQ + ch * CHUNK
                    pp = (pair * NCHUNKS + ch) % 2
                    for t in range(NTERMS):
                        lt = lts[t][pp]
                        nc.gpsimd.dma_start(
                            out=lt[0:rpad, :, 0:C],
                            in_=lhst_d[t, 0, :, g0:g0 + CHUNK, :])
                        nc.gpsimd.dma_start(
                            out=lt[rpad:2 * rpad, :, C:128],
                            in_=lhst_d[t, 1, :, g0:g0 + CHUNK, :])
                    st = stagep.tile([128, CHUNK, NX], f32)
                    for w in range(CHUNK):
                        mask = maskp.tile([K, NX], bf16)
                        nc.vector.tensor_scalar(
                            out=mask[:],
                            in0=meta[:, 0:NX],
                            scalar1=meta[:, NX + g0 + w:NX + g0 + w + 1],
                            scalar2=None,
                            op0=mybir.AluOpType.is_equal,
                        )
                        acc = psump.tile([128, NX], f32)
                        for t in range(NTERMS):
                            nc.tensor.matmul(
                                acc[:], lts[t][pp][:, w, :], mask[:],
                                start=(t == 0), stop=(t == NTERMS - 1))
                        if gw % DVE_COPY_EVERY == 0:
                            nc.vector.tensor_copy(out=st[:, w, :], in_=acc[:])
                        else:
                            nc.scalar.copy(st[:, w, :], acc[:])
                        gw += 1
                    nc.sync.dma_start(out=out_d[pair, ch], in_=st[:])
    nc.compile()
    return nc


def _split3(feats):
    """Split fp32 features into 3 bf16 terms summing exactly to the input."""
    h1 = feats.astype(ml_dtypes.bfloat16)
    r1 = feats - h1.astype(np.float32)
    h2 = r1.astype(ml_dtypes.bfloat16)
    h3 = (r1 - h2.astype(np.float32)).astype(ml_dtypes.bfloat16)
    rec = (h1.astype(np.float32) + h2.astype(np.float32)) + h3.astype(np.float32)
    if not (rec == feats).all():
        raise FloatingPointError("bf16 3-term split not exact")
    return h1, h2, h3


def _pack(inputs, rpad):
    lhst = np.zeros((NCORES, NTERMS, 2, rpad, WINDOWS, C), ml_dtypes.bfloat16)
    meta = np.zeros((NCORES, 128, NX + WINDOWS), np.float32)
    meta[:, :, 0:NX] = np.arange(NX, dtype=np.float32)[None, None, :]
    meta[:, :, NX:] = -1.0

    for bin_i in range(NBINS):
        feats = np.asarray(inputs[f"pillar_features_bin_{bin_i}"], np.float32)
        terms = _split3(feats)
        coords = np.asarray(inputs[f"voxel_coords_bin_{bin_i}"])
        cb = np.asarray(coords[:, 0], np.int64)
        cy = np.asarray(coords[:, 2], np.int64)
        cx = np.asarray(coords[:, 3], np.int64)
        for b in range(B):
            rows_b = np.nonzero(cb == b)[0]
            y_b, x_b = cy[rows_b], cx[rows_b]
            for yq in range(4):
                q = bin_i * 16 + b * 4 + yq
                core, j = divmod(q, QPC)
                pair, half = divmod(j, 2)
                sel = (y_b >= YQ * yq) & (y_b < YQ * (yq + 1))
                rows = rows_b[sel]
                yl = y_b[sel] - YQ * yq
                xs = x_b[sel]
                order = np.argsort(yl, kind="stable")
                rows, yl, xs = rows[order], yl[order], xs[order]
                cnt = np.bincount(yl, minlength=YQ)
                if cnt.max() > rpad:
                    raise OverflowError(int(cnt.max()))
                off = np.concatenate([[0], np.cumsum(cnt)[:-1]])
                slot = np.arange(len(rows)) - off[yl]
                wins = pair * YQ + yl
                for t in range(NTERMS):
                    lhst[core, t, half, slot, wins, :] = terms[t][rows]
                meta[core, slot + half * rpad, NX + wins] = xs.astype(np.float32)
    return [{"lhst": lhst[c], "meta": meta[c]} for c in range(NCORES)]


def _run(inputs, rpad, trace=False):
    if rpad not in _cache:
        _cache[rpad] = _build(rpad)
    nc = _cache[rpad]
    in_maps = _pack(inputs, rpad)
    res = run_bass_kernel_spmd(nc, in_maps, core_ids=list(range(NCORES)),
                               trace=trace)
    outs = [np.zeros((B, C, NY, NX), np.float32) for _ in range(NBINS)]
    for q in range(NQ):
        bin_i, rem = divmod(q, 16)
        b, yq = divmod(rem, 4)
        core, j = divmod(q, QPC)
        pair, half = divmod(j, 2)
        # [NCHUNKS, 64, CHUNK, NX] -> [64, NCHUNKS*CHUNK, NX]
        blk = res.results[core]["out"][pair, :, half * C:(half + 1) * C]
        outs[bin_i][b, :, YQ * yq:YQ * (yq + 1), :] = (
            blk.transpose(1, 0, 2, 3).reshape(C, YQ, NX))
    return tuple(outs), res


def kernel(**inputs):
    rpad = RPAD
    while True:
        try:
            out, _ = _run(inputs, rpad)
            return out
        except OverflowError as e:
            rpad = (int(e.args[0]) + 7) // 8 * 8


def kernel_traced(**inputs):
    """Like kernel() but also returns BassKernelResults (for test.py)."""
    return _run(inputs, RPAD, trace=True)
